# revision 1
# baseline (speedup 1.0000x reference)
"""Cross-graph attention kernel for Trainium2 (8 NeuronCores, SPMD data-parallel over B).

Problem (B=32 graphs, NA=NB=128 nodes, D=128):
    xa = ha @ W1a.T ; xb = hb @ W1b.T                      (per graph)
    scores[n,m] = sum_h relu(xa[n,h] + xb[m,h] + b1[h]) * w2[h]  (+ b2, which
                  cancels in both softmaxes and is dropped)
    mu_a = ha - softmax_m(scores) @ hb
    mu_b = hb - softmax_n(scores).T @ ha

Sharding: data-parallel over B across 8 cores (4 graphs/core), sim_net params
replicated. All pairwise intermediates stay in SBUF/PSUM.

Per-core kernel design (per graph g):
  - xa_T[h,n], xb_T[h,m] via PE matmuls (contraction dim d on partitions).
  - xb' = bf16(xb_T), xab = xa_T + b1 once per graph.
  - Per n: t_n = relu(xb' + xab[:,n]) [h=128 part, m=128 free] bf16, as ONE
    fused op: DVE tensor_scalar (op0=add per-partition scalar, op1=max 0)
    for ~2/3 of n, ACT activation(Relu, bias=...) for ~1/3 (both engines run
    phase 1 concurrently; the split matches their measured per-tile rates).
  - Scores via 32 PE matmuls per graph (not 128 matvecs): moving operand
    t4 = [t_q | t_{q+32} | t_{q+64} | t_{q+96}] [128, 512], stationary
    w2s[:, 32-q:160-q] = w2 (x) comb(p%32==q). Each matmul fills score rows
    {q, q+32, q+64, q+96} of one [128, 512] PSUM bank; with n = q + 32*j,
    row p's own scores sit at free block p//32, so no shuffle is needed.
  - exp via 4 partition-range ACT ops -> compact E[n,m] bf16 (no max
    subtraction: softmax is shift-invariant and scores are O(+-10)).
  - mu_b: lhsT=E[n,m], rhs=[ha | -1]_bf16 -> psum [m, 0:128]=num,
    col 128 = -S_ba; mu_b = hb + num * (1/-S) as one scalar_tensor_tensor.
  - mu_a: transpose E via PE -> E_T[m,n], symmetric with [hb | -1].
"""

import numpy as np
import ml_dtypes

import concourse.bass as bass
import concourse.tile as tile
from concourse import bacc, mybir
from concourse import bass_utils
from concourse.masks import make_identity

F32 = mybir.dt.float32
BF16 = mybir.dt.bfloat16
AF = mybir.ActivationFunctionType
OP = mybir.AluOpType

B, NA, NB, D = 32, 128, 128, 128
NCORES = 8
G = B // NCORES  # graphs per core

_CACHE = {}


def _build_program():
    nc = bacc.Bacc(
        "TRN2",
        target_bir_lowering=False,
        debug=False,
        enable_asserts=False,
        num_devices=NCORES,
    )

    # Per-core DRAM I/O. haE/hbE carry an extra last column == -1.0 so the
    # attention matmul also produces the (negated) softmax denominator.
    haE_d = nc.dram_tensor("haE", [G * NA, D + 1], F32, kind="ExternalInput")
    hbE_d = nc.dram_tensor("hbE", [G * NB, D + 1], F32, kind="ExternalInput")
    haEb_d = nc.dram_tensor("haEb", [G * NA, D + 1], BF16, kind="ExternalInput")
    hbEb_d = nc.dram_tensor("hbEb", [G * NB, D + 1], BF16, kind="ExternalInput")
    haT_d = nc.dram_tensor("haT", [D, G * NA], F32, kind="ExternalInput")
    hbT_d = nc.dram_tensor("hbT", [D, G * NB], F32, kind="ExternalInput")
    w1aT_d = nc.dram_tensor("w1aT", [D, D], F32, kind="ExternalInput")
    w1bT_d = nc.dram_tensor("w1bT", [D, D], F32, kind="ExternalInput")
    b1_d = nc.dram_tensor("b1c", [D, 1], F32, kind="ExternalInput")
    # Group-q stationary for the scores matmul: lhsT_q[h, p] = w2[h] *
    # (p % 32 == q). All 32 of them are column-shifts of one comb pattern,
    # stored once as w2s[h, c] = w2[h] * (c % 32 == 0), c in [0, 160);
    # lhsT_q = w2s[:, 32-q : 160-q]. One matmul per q contracts FOUR relu
    # tiles (moving [128, 512]) and writes score rows {q, q+32, q+64, q+96};
    # with the n = q + 32*j assignment, partition p's own score row lands at
    # free block p//32, so exp reads 4 contiguous partition ranges.
    w2s_d = nc.dram_tensor("w2s", [D, 160], BF16, kind="ExternalInput")
    mua_d = nc.dram_tensor("mu_a", [G * NA, D], F32, kind="ExternalOutput")
    mub_d = nc.dram_tensor("mu_b", [G * NB, D], F32, kind="ExternalOutput")

    haE = haE_d.ap().rearrange("(g n) c -> g n c", g=G)
    hbE = hbE_d.ap().rearrange("(g n) c -> g n c", g=G)
    haT = haT_d.ap()
    hbT = hbT_d.ap()
    mua = mua_d.ap().rearrange("(g n) c -> g n c", g=G)
    mub = mub_d.ap().rearrange("(g n) c -> g n c", g=G)

    with tile.TileContext(nc) as tc:
        with (
            tc.tile_pool(name="consts", bufs=1) as consts,
            tc.tile_pool(name="io", bufs=3) as io,
            tc.tile_pool(name="xa", bufs=2) as xa_pool,
            tc.tile_pool(name="xbp", bufs=2) as xbp_pool,
            tc.tile_pool(name="t", bufs=8) as t_pool,
            tc.tile_pool(name="ee", bufs=2) as e_pool,
            tc.tile_pool(name="r", bufs=4) as r_pool,
            tc.tile_pool(name="outs", bufs=4) as out_pool,
            tc.tile_pool(name="prep_ps", bufs=2, space="PSUM") as prep_ps,
            tc.tile_pool(name="sc_ps", bufs=2, space="PSUM") as sc_ps,
            tc.tile_pool(name="tr_ps", bufs=1, space="PSUM") as tr_ps,
            tc.tile_pool(name="ab_ps", bufs=3, space="PSUM") as ab_ps,
        ):
            ident_bf = consts.tile([128, 128], BF16)
            make_identity(nc, ident_bf)
            w1aT_sb = consts.tile([D, D], F32)
            nc.sync.dma_start(out=w1aT_sb, in_=w1aT_d.ap())
            w1bT_sb = consts.tile([D, D], F32)
            nc.sync.dma_start(out=w1bT_sb, in_=w1bT_d.ap())
            b1_sb = consts.tile([D, 1], F32)
            nc.sync.dma_start(out=b1_sb, in_=b1_d.ap())
            w2s_sb = consts.tile([D, 160], BF16)
            nc.sync.dma_start(out=w2s_sb, in_=w2s_d.ap())

            haEb = haEb_d.ap().rearrange("(g n) c -> g n c", g=G)
            hbEb = hbEb_d.ap().rearrange("(g n) c -> g n c", g=G)

            for g in range(G):
                # haT/hbT first: they gate the prep matmuls (phase-1 critical
                # path); the haE/hbE family is only needed in phase 2.
                haT_sb = io.tile([D, NA], F32, tag="haT")
                nc.sync.dma_start(out=haT_sb, in_=haT[:, g * NA:(g + 1) * NA])
                hbT_sb = io.tile([D, NB], F32, tag="hbT")
                nc.sync.dma_start(out=hbT_sb, in_=hbT[:, g * NB:(g + 1) * NB])
                haE_sb = io.tile([NA, D + 1], F32, tag="haE")
                nc.sync.dma_start(out=haE_sb, in_=haE[g])
                hbE_sb = io.tile([NB, D + 1], F32, tag="hbE")
                nc.sync.dma_start(out=hbE_sb, in_=hbE[g])
                haEb_sb = io.tile([NA, D + 1], BF16, tag="haEb")
                nc.sync.dma_start(out=haEb_sb, in_=haEb[g])
                hbEb_sb = io.tile([NB, D + 1], BF16, tag="hbEb")
                nc.sync.dma_start(out=hbEb_sb, in_=hbEb[g])

                # xa_T[h, n] = W1a @ ha^T ; xb_T[h, m] = W1b @ hb^T  (fp32)
                ps_xa = prep_ps.tile([D, NA], F32, tag="prep")
                nc.tensor.matmul(ps_xa, lhsT=w1aT_sb, rhs=haT_sb, start=True, stop=True)
                # xab = xa + b1 (per-n bias column source for both engines)
                xab_sb = xa_pool.tile([D, NA], F32, tag="xa")
                nc.vector.tensor_scalar(
                    out=xab_sb, in0=ps_xa, scalar1=b1_sb[:, 0:1], scalar2=None,
                    op0=OP.add,
                )

                # ps_xb stays resident in PSUM all of phase 1: the ACT relu
                # path reads it directly (PSUM-source ACT is faster than SBUF).
                ps_xb = prep_ps.tile([D, NB], F32, tag="prep")
                nc.tensor.matmul(ps_xb, lhsT=w1bT_sb, rhs=hbT_sb, start=True, stop=True)
                xb_bf = xbp_pool.tile([D, NB], BF16, tag="xbp")
                nc.vector.tensor_copy(out=xb_bf, in_=ps_xb)

                # Phase 1: t4 = [t_{q} | t_{q+32} | t_{q+64} | t_{q+96}]
                # (t_n = relu(xb + xa_n + b1), [h, m] bf16), one matmul per q
                # with the group-q stationary accumulating all scores in psum.
                ps_sc = sc_ps.tile([NA, 4 * NB], F32, tag="sc")
                for q in range(32):
                    t4 = t_pool.tile([D, 4 * NB], BF16, tag="t")
                    for j in range(4):
                        n = q + 32 * j
                        ts = t4[:, j * NB:(j + 1) * NB]
                        if n % 14 in (0, 3, 6, 9, 12):
                            nc.scalar.activation(
                                out=ts, in_=xb_bf, func=AF.Relu,
                                bias=xab_sb[:, n:n + 1], scale=1.0,
                            )
                        else:
                            nc.vector.tensor_scalar(
                                out=ts, in0=xb_bf,
                                scalar1=xab_sb[:, n:n + 1], scalar2=0.0,
                                op0=OP.add, op1=OP.max,
                            )
                    nc.tensor.matmul(
                        ps_sc, lhsT=w2s_sb[:, 32 - q:160 - q], rhs=t4,
                        start=(q == 0), stop=(q == 31),
                    )

                # E[n, m] = exp(scores): partition range [32u, 32u+32) holds its
                # own scores at free block u.
                e_sb = e_pool.tile([NA, NB], BF16, tag="E")
                for u in range(4):
                    nc.scalar.activation(
                        out=e_sb[32 * u:32 * (u + 1), :],
                        in_=ps_sc[32 * u:32 * (u + 1), u * NB:(u + 1) * NB],
                        func=AF.Exp,
                    )

                # E_T[m, n] via PE transpose
                ps_tr = tr_ps.tile([NB, NA], BF16, tag="tr")
                nc.tensor.transpose(ps_tr, e_sb, ident_bf)
                et_sb = e_pool.tile([NB, NA], BF16, tag="Et")
                nc.scalar.copy(out=et_sb, in_=ps_tr)

                # num_a[n, 0:128], -S_ab[n] at col 128
                ps_a = ab_ps.tile([NA, D + 1], F32, tag="ab")
                nc.tensor.matmul(ps_a, lhsT=et_sb, rhs=hbEb_sb, start=True, stop=True)
                # num_b[m, 0:128], -S_ba[m] at col 128
                ps_b = ab_ps.tile([NB, D + 1], F32, tag="ab")
                nc.tensor.matmul(ps_b, lhsT=e_sb, rhs=haEb_sb, start=True, stop=True)

                ra = r_pool.tile([NA, 1], F32, tag="r")
                nc.vector.reciprocal(out=ra, in_=ps_a[:, D:D + 1])
                outa = out_pool.tile([NA, D], F32, tag="oa")
                # mu_a = ha + num_a * (-1/S_ab)
                nc.vector.scalar_tensor_tensor(
                    out=outa, in0=ps_a[:, 0:D], scalar=ra[:, 0:1],
                    in1=haE_sb[:, 0:D], op0=OP.mult, op1=OP.add,
                )
                nc.sync.dma_start(out=mua[g], in_=outa)

                rb = r_pool.tile([NB, 1], F32, tag="r")
                nc.vector.reciprocal(out=rb, in_=ps_b[:, D:D + 1])
                outb = out_pool.tile([NB, D], F32, tag="ob")
                nc.vector.scalar_tensor_tensor(
                    out=outb, in0=ps_b[:, 0:D], scalar=rb[:, 0:1],
                    in1=hbE_sb[:, 0:D], op0=OP.mult, op1=OP.add,
                )
                nc.sync.dma_start(out=mub[g], in_=outb)

    nc.compile()
    return nc


def _get_program():
    if "nc" not in _CACHE:
        _CACHE["nc"] = _build_program()
    return _CACHE["nc"]


def _prep_in_maps(h_a, h_b, W1, b1, W2):
    h_a = np.asarray(h_a, dtype=np.float32)
    h_b = np.asarray(h_b, dtype=np.float32)
    W1 = np.asarray(W1, dtype=np.float32)
    b1 = np.asarray(b1, dtype=np.float32)
    W2 = np.asarray(W2, dtype=np.float32)

    # W1a[h, d] = W1[h, d], W1b[h, d] = W1[h, D + d]; lhsT wants [d, h].
    w1aT = np.ascontiguousarray(W1[:, :D].T)
    w1bT = np.ascontiguousarray(W1[:, D:].T)
    b1c = np.ascontiguousarray(b1.reshape(D, 1))
    w2bf = W2[0].astype(ml_dtypes.bfloat16).astype(np.float32)
    comb = (np.arange(160) % 32 == 0).astype(np.float32)
    w2s = np.ascontiguousarray(w2bf[:, None] * comb[None, :]).astype(ml_dtypes.bfloat16)

    neg = np.full((G * NA, 1), -1.0, dtype=np.float32)

    in_maps = []
    for c in range(NCORES):
        ha = h_a[c * G * NA:(c + 1) * G * NA]  # [G*NA, D]
        hb = h_b[c * G * NB:(c + 1) * G * NB]
        haE = np.ascontiguousarray(np.concatenate([ha, neg], axis=1))
        hbE = np.ascontiguousarray(np.concatenate([hb, neg], axis=1))
        haT = np.ascontiguousarray(
            ha.reshape(G, NA, D).transpose(2, 0, 1).reshape(D, G * NA))
        hbT = np.ascontiguousarray(
            hb.reshape(G, NB, D).transpose(2, 0, 1).reshape(D, G * NB))
        in_maps.append({
            "haE": haE, "hbE": hbE, "haT": haT, "hbT": hbT,
            "haEb": haE.astype(ml_dtypes.bfloat16),
            "hbEb": hbE.astype(ml_dtypes.bfloat16),
            "w1aT": w1aT, "w1bT": w1bT, "b1c": b1c, "w2s": w2s,
        })
    return in_maps


def run(h_a, h_b, W1, b1, W2, trace=False, **run_kwargs):
    nc = _get_program()
    in_maps = _prep_in_maps(h_a, h_b, W1, b1, W2)
    res = bass_utils.run_bass_kernel_spmd(
        nc, in_maps, core_ids=list(range(NCORES)), trace=trace, **run_kwargs
    )
    mu_a = np.concatenate([r["mu_a"] for r in res.results], axis=0)
    mu_b = np.concatenate([r["mu_b"] for r in res.results], axis=0)
    return (mu_a, mu_b), res


def kernel(h_a, batch_a, h_b, batch_b, W1, b1, W2, b2):
    # batch_a/batch_b encode the (equal-sized, sorted) graph partition that the
    # dense [B, n, D] view already assumes; b2 shifts scores uniformly and
    # cancels in both softmaxes.
    (mu_a, mu_b), _ = run(h_a, h_b, W1, b1, W2, trace=False)
    return mu_a, mu_b



# revision 6
# speedup vs baseline: 1.0326x; 1.0326x over previous
"""Cross-graph attention kernel for Trainium2 (8 NeuronCores, SPMD data-parallel over B).

v2: three-way engine balance. Per graph (B=32, NA=NB=D=128):
  - prep (bf16 PE): xa_T[h,n], xa_rows[n,h], xb_T[h,m]; xb1 = xb_T + b1 (bf16),
    xa_sb (f32 scalar source), xar_bf (bf16 stationary).
  - relu tensor t[n][h,m] = relu(xb1 + xa_n) produced by TWO lanes:
    * DVE lane (q in QD): 4x tensor_scalar(add col, max 0) [128,128] bf16
      -> one bf16 comb matmul per quad into the scores psum (diagonal layout).
    * ACT lane (q pairs (q, q+16)): PE builds y4 = xb1(x4) + xa rows in PSUM
      (identity stationary with broadcast moving + xar stationary with
      stride-0 comb moving over identity columns), ACT applies relu
      PSUM->fp8 [128,512]; two quads -> one fp8 DoubleRow matmul (2x K) into
      the same scores psum.
  - scores psum [128,512]: row p holds its scores at free block p//32.
  - exp via 4 partition-range ACT ops -> E bf16; transpose via PE;
    attention matmuls with [h|-1]-extended hb/ha (bf16) give numerator and
    -denominator; mu = h + num * (-1/S) as scalar_tensor_tensor.
"""

import numpy as np
import ml_dtypes

import concourse.bass as bass
import concourse.tile as tile
from concourse import bacc, mybir
from concourse import bass_utils
from concourse.masks import make_identity

F32 = mybir.dt.float32
BF16 = mybir.dt.bfloat16
FP8 = mybir.dt.float8e4
AF = mybir.ActivationFunctionType
OP = mybir.AluOpType
DRM = mybir.MatmulPerfMode.DoubleRow

B, NA, NB, D = 32, 128, 128, 128
NCORES = 8
G = B // NCORES  # graphs per core

# per-graph lane split: even graphs 8 ACT pairs, odd graphs 7 (DVE/ACT
# balance ~68/60 quads per core)
PAIRS = {0: 8, 1: 7, 2: 8, 3: 7}

_CACHE = {}


def _build_program():
    nc = bacc.Bacc(
        "TRN2",
        target_bir_lowering=False,
        debug=False,
        enable_asserts=False,
        num_devices=NCORES,
    )

    haE_d = nc.dram_tensor("haE", [G * NA, D + 1], F32, kind="ExternalInput")
    hbE_d = nc.dram_tensor("hbE", [G * NB, D + 1], F32, kind="ExternalInput")
    haEb_d = nc.dram_tensor("haEb", [G * NA, D + 1], BF16, kind="ExternalInput")
    hbEb_d = nc.dram_tensor("hbEb", [G * NB, D + 1], BF16, kind="ExternalInput")
    haT_d = nc.dram_tensor("haT", [D, G * NA], BF16, kind="ExternalInput")
    hbT_d = nc.dram_tensor("hbT", [D, G * NB], BF16, kind="ExternalInput")
    w1aT_d = nc.dram_tensor("w1aT", [D, D], BF16, kind="ExternalInput")
    w1bT_d = nc.dram_tensor("w1bT", [D, D], BF16, kind="ExternalInput")
    b1_d = nc.dram_tensor("b1c", [D, 1], F32, kind="ExternalInput")
    w2s_d = nc.dram_tensor("w2s", [D, 160], BF16, kind="ExternalInput")
    w8_d = nc.dram_tensor("w8", [D, 8 * 256], FP8, kind="ExternalInput")
    mua_d = nc.dram_tensor("mu_a", [G * NA, D], F32, kind="ExternalOutput")
    mub_d = nc.dram_tensor("mu_b", [G * NB, D], F32, kind="ExternalOutput")

    haE = haE_d.ap().rearrange("(g n) c -> g n c", g=G)
    hbE = hbE_d.ap().rearrange("(g n) c -> g n c", g=G)
    haT = haT_d.ap()
    hbT = hbT_d.ap()
    haEb = haEb_d.ap().rearrange("(g n) c -> g n c", g=G)
    hbEb = hbEb_d.ap().rearrange("(g n) c -> g n c", g=G)
    mua = mua_d.ap().rearrange("(g n) c -> g n c", g=G)
    mub = mub_d.ap().rearrange("(g n) c -> g n c", g=G)

    with tile.TileContext(nc) as tc:
        with (
            tc.tile_pool(name="consts", bufs=1) as consts,
            tc.tile_pool(name="io", bufs=3) as io,
            tc.tile_pool(name="gp", bufs=2) as gp,
            tc.tile_pool(name="t", bufs=8) as t_pool,
            tc.tile_pool(name="y8p", bufs=3) as y8_pool,
            tc.tile_pool(name="ee", bufs=2) as e_pool,
            tc.tile_pool(name="r", bufs=4) as r_pool,
            tc.tile_pool(name="outs", bufs=4) as out_pool,
            tc.tile_pool(name="prep_ps", bufs=1, space="PSUM") as prep_ps,
            tc.tile_pool(name="y4_ps", bufs=2, space="PSUM") as y4_ps,
            tc.tile_pool(name="sc_ps", bufs=2, space="PSUM") as sc_ps,
            tc.tile_pool(name="tr_ps", bufs=1, space="PSUM") as tr_ps,
            tc.tile_pool(name="ab_ps", bufs=1, space="PSUM") as ab_ps,
        ):
            ident_bf = consts.tile([128, 128], BF16)
            make_identity(nc, ident_bf)
            w1aT_sb = consts.tile([D, D], BF16)
            nc.sync.dma_start(out=w1aT_sb, in_=w1aT_d.ap())
            w1bT_sb = consts.tile([D, D], BF16)
            nc.sync.dma_start(out=w1bT_sb, in_=w1bT_d.ap())
            b1_sb = consts.tile([D, 1], F32)
            nc.sync.dma_start(out=b1_sb, in_=b1_d.ap())
            w2s_sb = consts.tile([D, 160], BF16)
            nc.sync.dma_start(out=w2s_sb, in_=w2s_d.ap())
            w8_sb = consts.tile([D, 8 * 256], FP8)
            nc.sync.dma_start(out=w8_sb, in_=w8_d.ap())

            for g in range(G):
                # phase-1-critical DMAs first
                haT_sb = io.tile([D, NA], BF16, tag="haT")
                nc.sync.dma_start(out=haT_sb, in_=haT[:, g * NA:(g + 1) * NA])
                hbT_sb = io.tile([D, NB], BF16, tag="hbT")
                nc.sync.dma_start(out=hbT_sb, in_=hbT[:, g * NB:(g + 1) * NB])
                haE_sb = io.tile([NA, D + 1], F32, tag="haE")
                nc.sync.dma_start(out=haE_sb, in_=haE[g])
                hbE_sb = io.tile([NB, D + 1], F32, tag="hbE")
                nc.sync.dma_start(out=hbE_sb, in_=hbE[g])
                haEb_sb = io.tile([NA, D + 1], BF16, tag="haEb")
                nc.sync.dma_start(out=haEb_sb, in_=haEb[g])
                hbEb_sb = io.tile([NB, D + 1], BF16, tag="hbEb")
                nc.sync.dma_start(out=hbEb_sb, in_=hbEb[g])

                # prep matmuls (bf16)
                ps_prep = prep_ps.tile([D, 384], F32, tag="prep")
                ps_xa = ps_prep[:, 0:128]
                nc.tensor.matmul(ps_xa, lhsT=w1aT_sb, rhs=haT_sb, start=True, stop=True)
                ps_xar = ps_prep[:, 128:256]
                nc.tensor.matmul(ps_xar, lhsT=haT_sb, rhs=w1aT_sb, start=True, stop=True)
                ps_xb = ps_prep[:, 256:384]
                nc.tensor.matmul(ps_xb, lhsT=w1bT_sb, rhs=hbT_sb, start=True, stop=True)

                xa_sb = gp.tile([D, NA], F32, tag="xa")
                nc.vector.tensor_copy(out=xa_sb, in_=ps_xa)
                xb1_bf = gp.tile([D, NB], BF16, tag="xb1")
                nc.vector.tensor_scalar(
                    out=xb1_bf, in0=ps_xb, scalar1=b1_sb[:, 0:1], scalar2=None,
                    op0=OP.add,
                )
                xar_bf = gp.tile([NA, D], BF16, tag="xar")
                nc.scalar.copy(out=xar_bf, in_=ps_xar)

                xb1_b = xb1_bf[:, :].unsqueeze(1).broadcast_to((128, 4, 128))

                npairs = PAIRS[g]
                qd = [q for q in range(8, 16)] + [q for q in range(24, 32)]
                if npairs == 7:
                    qd = [7] + qd[:8] + [23] + qd[8:]
                # build emission plan: per pair, its two ACT quads, then some
                # DVE quads, then the pair's DR matmul
                plan = []
                di = 0
                for i in range(npairs):
                    plan.append(("ar", i))  # ACT relu pair i (both quads)
                    take = round((i + 1) * len(qd) / npairs) - di
                    for _ in range(take):
                        plan.append(("d", qd[di]))
                        di += 1
                    plan.append(("drmm", i))

                n_sc_mms = len(qd) + npairs
                ps_sc = sc_ps.tile([NA, 4 * NB], F32, tag="sc")
                sc_i = 0

                y8_tiles = {}
                for kind, v in plan:
                    if kind == "ar":
                        i = v
                        y8 = y8_pool.tile([128, 1024], FP8, tag="y8")
                        y8_tiles[i] = y8
                        for half, q in ((0, i), (1, i + 16)):
                            ps_y4 = y4_ps.tile([128, 512], F32, tag="y4")
                            nc.tensor.matmul(
                                ps_y4, lhsT=ident_bf, rhs=xb1_b,
                                start=True, stop=False, skip_group_check=True)
                            comb = ident_bf[:, q:q + 97:32].unsqueeze(2) \
                                .broadcast_to((128, 4, 128))
                            nc.tensor.matmul(
                                ps_y4, lhsT=xar_bf, rhs=comb,
                                start=False, stop=True, skip_group_check=True)
                            nc.scalar.activation(
                                out=y8[:, half * 512:(half + 1) * 512],
                                in_=ps_y4, func=AF.Relu)
                    elif kind == "d":
                        q = v
                        t4 = t_pool.tile([D, 4 * NB], BF16, tag="t")
                        for j in range(4):
                            n = q + 32 * j
                            nc.vector.tensor_scalar(
                                out=t4[:, j * NB:(j + 1) * NB], in0=xb1_bf,
                                scalar1=xa_sb[:, n:n + 1], scalar2=0.0,
                                op0=OP.add, op1=OP.max,
                            )
                        nc.tensor.matmul(
                            ps_sc, lhsT=w2s_sb[:, 32 - q:160 - q], rhs=t4,
                            start=(sc_i == 0), stop=(sc_i == n_sc_mms - 1),
                            skip_group_check=True,
                        )
                        sc_i += 1
                    else:  # drmm
                        i = v
                        y8 = y8_tiles.pop(i)
                        w83 = w8_sb[:, i * 256:(i + 1) * 256].rearrange(
                            "p (two m) -> p two m", two=2)
                        y83 = y8.rearrange("p (two n) -> p two n", two=2)
                        nc.tensor.matmul(
                            ps_sc, lhsT=w83, rhs=y83, perf_mode=DRM,
                            start=(sc_i == 0), stop=(sc_i == n_sc_mms - 1),
                            skip_group_check=True,
                        )
                        sc_i += 1

                # E[n, m] = exp(scores)
                e_sb = e_pool.tile([NA, NB], BF16, tag="E")
                for u in range(4):
                    nc.scalar.activation(
                        out=e_sb[32 * u:32 * (u + 1), :],
                        in_=ps_sc[32 * u:32 * (u + 1), u * NB:(u + 1) * NB],
                        func=AF.Exp,
                    )

                ps_tr = tr_ps.tile([NB, NA], BF16, tag="tr")
                nc.tensor.transpose(ps_tr, e_sb, ident_bf)
                et_sb = e_pool.tile([NB, NA], BF16, tag="Et")
                nc.scalar.copy(out=et_sb, in_=ps_tr)

                ps_ab = ab_ps.tile([NA, 272], F32, tag="ab")
                ps_a = ps_ab[:, 0:129]
                nc.tensor.matmul(ps_a, lhsT=et_sb, rhs=hbEb_sb, start=True, stop=True)
                ps_b = ps_ab[:, 136:265]
                nc.tensor.matmul(ps_b, lhsT=e_sb, rhs=haEb_sb, start=True, stop=True)

                ra = r_pool.tile([NA, 1], F32, tag="r")
                nc.vector.reciprocal(out=ra, in_=ps_a[:, D:D + 1])
                outa = out_pool.tile([NA, D], F32, tag="oa")
                nc.vector.scalar_tensor_tensor(
                    out=outa, in0=ps_a[:, 0:D], scalar=ra[:, 0:1],
                    in1=haE_sb[:, 0:D], op0=OP.mult, op1=OP.add,
                )
                nc.sync.dma_start(out=mua[g], in_=outa)

                rb = r_pool.tile([NB, 1], F32, tag="r")
                nc.vector.reciprocal(out=rb, in_=ps_b[:, D:D + 1])
                outb = out_pool.tile([NB, D], F32, tag="ob")
                nc.vector.scalar_tensor_tensor(
                    out=outb, in0=ps_b[:, 0:D], scalar=rb[:, 0:1],
                    in1=hbE_sb[:, 0:D], op0=OP.mult, op1=OP.add,
                )
                nc.sync.dma_start(out=mub[g], in_=outb)

    nc.compile()
    return nc


def _get_program():
    if "nc" not in _CACHE:
        _CACHE["nc"] = _build_program()
    return _CACHE["nc"]


def _prep_in_maps(h_a, h_b, W1, b1, W2):
    h_a = np.asarray(h_a, dtype=np.float32)
    h_b = np.asarray(h_b, dtype=np.float32)
    W1 = np.asarray(W1, dtype=np.float32)
    b1 = np.asarray(b1, dtype=np.float32)
    W2 = np.asarray(W2, dtype=np.float32)

    w1aT = np.ascontiguousarray(W1[:, :D].T).astype(ml_dtypes.bfloat16)
    w1bT = np.ascontiguousarray(W1[:, D:].T).astype(ml_dtypes.bfloat16)
    b1c = np.ascontiguousarray(b1.reshape(D, 1))
    w2bf = W2[0].astype(ml_dtypes.bfloat16).astype(np.float32)
    comb = (np.arange(160) % 32 == 0).astype(np.float32)
    w2s_f = w2bf[:, None] * comb[None, :]
    w2s = np.ascontiguousarray(w2s_f).astype(ml_dtypes.bfloat16)
    # DR stationaries: pair i -> slot0 = comb shift for q=i, slot1 for q=i+16
    w8 = np.concatenate(
        [np.concatenate([w2s_f[:, 32 - i:160 - i], w2s_f[:, 16 - i:144 - i]],
                        axis=1) for i in range(8)], axis=1)
    w8 = np.ascontiguousarray(w8).astype(ml_dtypes.float8_e4m3)

    neg = np.full((G * NA, 1), -1.0, dtype=np.float32)

    in_maps = []
    for c in range(NCORES):
        ha = h_a[c * G * NA:(c + 1) * G * NA]
        hb = h_b[c * G * NB:(c + 1) * G * NB]
        haE = np.ascontiguousarray(np.concatenate([ha, neg], axis=1))
        hbE = np.ascontiguousarray(np.concatenate([hb, neg], axis=1))
        haT = np.ascontiguousarray(
            ha.reshape(G, NA, D).transpose(2, 0, 1).reshape(D, G * NA)
        ).astype(ml_dtypes.bfloat16)
        hbT = np.ascontiguousarray(
            hb.reshape(G, NB, D).transpose(2, 0, 1).reshape(D, G * NB)
        ).astype(ml_dtypes.bfloat16)
        in_maps.append({
            "haE": haE, "hbE": hbE, "haT": haT, "hbT": hbT,
            "haEb": haE.astype(ml_dtypes.bfloat16),
            "hbEb": hbE.astype(ml_dtypes.bfloat16),
            "w1aT": w1aT, "w1bT": w1bT, "b1c": b1c, "w2s": w2s, "w8": w8,
        })
    return in_maps


def run(h_a, h_b, W1, b1, W2, trace=False, **run_kwargs):
    nc = _get_program()
    in_maps = _prep_in_maps(h_a, h_b, W1, b1, W2)
    res = bass_utils.run_bass_kernel_spmd(
        nc, in_maps, core_ids=list(range(NCORES)), trace=trace, **run_kwargs
    )
    mu_a = np.concatenate([r["mu_a"] for r in res.results], axis=0)
    mu_b = np.concatenate([r["mu_b"] for r in res.results], axis=0)
    return (mu_a, mu_b), res


def kernel(h_a, batch_a, h_b, batch_b, W1, b1, W2, b2):
    # batch_a/batch_b encode the (equal-sized, sorted) graph partition that the
    # dense [B, n, D] view already assumes; b2 shifts scores uniformly and
    # cancels in both softmaxes.
    (mu_a, mu_b), _ = run(h_a, h_b, W1, b1, W2, trace=False)
    return mu_a, mu_b


# revision 7
# speedup vs baseline: 1.0929x; 1.0584x over previous
"""Cross-graph attention kernel for Trainium2 (8 NeuronCores, SPMD data-parallel over B).

v2: three-way engine balance. Per graph (B=32, NA=NB=D=128):
  - prep (bf16 PE): xa_T[h,n], xa_rows[n,h], xb_T[h,m]; xb1 = xb_T + b1 (bf16),
    xa_sb (f32 scalar source), xar_bf (bf16 stationary).
  - relu tensor t[n][h,m] = relu(xb1 + xa_n) produced by TWO lanes:
    * DVE lane (q in QD): 4x tensor_scalar(add col, max 0) [128,128] bf16
      -> one bf16 comb matmul per quad into the scores psum (diagonal layout).
    * ACT lane (q pairs (q, q+16)): PE builds y4 = xb1(x4) + xa rows in PSUM
      (identity stationary with broadcast moving + xar stationary with
      stride-0 comb moving over identity columns), ACT applies relu
      PSUM->fp8 [128,512]; two quads -> one fp8 DoubleRow matmul (2x K) into
      the same scores psum.
  - scores psum [128,512]: row p holds its scores at free block p//32.
  - exp via 4 partition-range ACT ops -> E bf16; transpose via PE;
    attention matmuls with [h|-1]-extended hb/ha (bf16) give numerator and
    -denominator; mu = h + num * (-1/S) as scalar_tensor_tensor.
"""

import numpy as np
import ml_dtypes

import concourse.bass as bass
import concourse.tile as tile
from concourse import bacc, mybir
from concourse import bass_utils
from concourse.masks import make_identity

F32 = mybir.dt.float32
BF16 = mybir.dt.bfloat16
FP8 = mybir.dt.float8e4
AF = mybir.ActivationFunctionType
OP = mybir.AluOpType
DRM = mybir.MatmulPerfMode.DoubleRow

B, NA, NB, D = 32, 128, 128, 128
NCORES = 8
G = B // NCORES  # graphs per core

# per-graph lane split: even graphs 8 ACT pairs, odd graphs 7 (DVE/ACT
# balance ~68/60 quads per core)
PAIRS = {0: 8, 1: 7, 2: 8, 3: 7}

_CACHE = {}


def _build_program():
    nc = bacc.Bacc(
        "TRN2",
        target_bir_lowering=False,
        debug=False,
        enable_asserts=False,
        num_devices=NCORES,
    )

    haE_d = nc.dram_tensor("haE", [G * NA, D + 1], F32, kind="ExternalInput")
    hbE_d = nc.dram_tensor("hbE", [G * NB, D + 1], F32, kind="ExternalInput")
    haEb_d = nc.dram_tensor("haEb", [G * NA, D + 1], BF16, kind="ExternalInput")
    hbEb_d = nc.dram_tensor("hbEb", [G * NB, D + 1], BF16, kind="ExternalInput")
    haT_d = nc.dram_tensor("haT", [D, G * NA], BF16, kind="ExternalInput")
    hbT_d = nc.dram_tensor("hbT", [D, G * NB], BF16, kind="ExternalInput")
    w1aT_d = nc.dram_tensor("w1aT", [D, D], BF16, kind="ExternalInput")
    w1bT_d = nc.dram_tensor("w1bT", [D, D], BF16, kind="ExternalInput")
    b1_d = nc.dram_tensor("b1c", [D, 1], F32, kind="ExternalInput")
    w2s_d = nc.dram_tensor("w2s", [D, 160], BF16, kind="ExternalInput")
    w8_d = nc.dram_tensor("w8", [D, 8 * 256], FP8, kind="ExternalInput")
    mua_d = nc.dram_tensor("mu_a", [G * NA, D], F32, kind="ExternalOutput")
    mub_d = nc.dram_tensor("mu_b", [G * NB, D], F32, kind="ExternalOutput")

    haE = haE_d.ap().rearrange("(g n) c -> g n c", g=G)
    hbE = hbE_d.ap().rearrange("(g n) c -> g n c", g=G)
    haT = haT_d.ap()
    hbT = hbT_d.ap()
    haEb = haEb_d.ap().rearrange("(g n) c -> g n c", g=G)
    hbEb = hbEb_d.ap().rearrange("(g n) c -> g n c", g=G)
    mua = mua_d.ap().rearrange("(g n) c -> g n c", g=G)
    mub = mub_d.ap().rearrange("(g n) c -> g n c", g=G)

    with tile.TileContext(nc) as tc:
        with (
            tc.tile_pool(name="consts", bufs=1) as consts,
            tc.tile_pool(name="io", bufs=3) as io,
            tc.tile_pool(name="gp", bufs=2) as gp,
            tc.tile_pool(name="t", bufs=8) as t_pool,
            tc.tile_pool(name="y8p", bufs=3) as y8_pool,
            tc.tile_pool(name="ee", bufs=2) as e_pool,
            tc.tile_pool(name="r", bufs=4) as r_pool,
            tc.tile_pool(name="outs", bufs=4) as out_pool,
            tc.tile_pool(name="prep_ps", bufs=1, space="PSUM") as prep_ps,
            tc.tile_pool(name="y4_ps", bufs=3, space="PSUM") as y4_ps,
            tc.tile_pool(name="sc_ps", bufs=2, space="PSUM") as sc_ps,
            tc.tile_pool(name="tr_ps", bufs=1, space="PSUM") as tr_ps,
            tc.tile_pool(name="ab_ps", bufs=1, space="PSUM") as ab_ps,
        ):
            ident_bf = consts.tile([128, 128], BF16)
            make_identity(nc, ident_bf)
            w1aT_sb = consts.tile([D, D], BF16)
            nc.sync.dma_start(out=w1aT_sb, in_=w1aT_d.ap())
            w1bT_sb = consts.tile([D, D], BF16)
            nc.sync.dma_start(out=w1bT_sb, in_=w1bT_d.ap())
            b1_sb = consts.tile([D, 1], F32)
            nc.sync.dma_start(out=b1_sb, in_=b1_d.ap())
            w2s_sb = consts.tile([D, 160], BF16)
            nc.sync.dma_start(out=w2s_sb, in_=w2s_d.ap())
            w8_sb = consts.tile([D, 8 * 256], FP8)
            nc.sync.dma_start(out=w8_sb, in_=w8_d.ap())

            for g in range(G):
                # phase-1-critical DMAs first
                haT_sb = io.tile([D, NA], BF16, tag="haT")
                nc.sync.dma_start(out=haT_sb, in_=haT[:, g * NA:(g + 1) * NA])
                hbT_sb = io.tile([D, NB], BF16, tag="hbT")
                nc.sync.dma_start(out=hbT_sb, in_=hbT[:, g * NB:(g + 1) * NB])
                haE_sb = io.tile([NA, D + 1], F32, tag="haE")
                nc.sync.dma_start(out=haE_sb, in_=haE[g])
                hbE_sb = io.tile([NB, D + 1], F32, tag="hbE")
                nc.sync.dma_start(out=hbE_sb, in_=hbE[g])
                haEb_sb = io.tile([NA, D + 1], BF16, tag="haEb")
                nc.sync.dma_start(out=haEb_sb, in_=haEb[g])
                hbEb_sb = io.tile([NB, D + 1], BF16, tag="hbEb")
                nc.sync.dma_start(out=hbEb_sb, in_=hbEb[g])

                # prep matmuls (bf16)
                ps_prep = prep_ps.tile([D, 384], F32, tag="prep")
                ps_xa = ps_prep[:, 0:128]
                nc.tensor.matmul(ps_xa, lhsT=w1aT_sb, rhs=haT_sb, start=True, stop=True)
                ps_xar = ps_prep[:, 128:256]
                nc.tensor.matmul(ps_xar, lhsT=haT_sb, rhs=w1aT_sb, start=True, stop=True)
                ps_xb = ps_prep[:, 256:384]
                nc.tensor.matmul(ps_xb, lhsT=w1bT_sb, rhs=hbT_sb, start=True, stop=True)

                xa_sb = gp.tile([D, NA], F32, tag="xa")
                nc.vector.tensor_copy(out=xa_sb, in_=ps_xa)
                xb1_bf = gp.tile([D, NB], BF16, tag="xb1")
                nc.vector.tensor_scalar(
                    out=xb1_bf, in0=ps_xb, scalar1=b1_sb[:, 0:1], scalar2=None,
                    op0=OP.add,
                )
                xar_bf = gp.tile([NA, D], BF16, tag="xar")
                nc.scalar.copy(out=xar_bf, in_=ps_xar)

                xb1_b = xb1_bf[:, :].unsqueeze(1).broadcast_to((128, 4, 128))

                npairs = PAIRS[g]
                qd = [q for q in range(8, 16)] + [q for q in range(24, 32)]
                if npairs == 7:
                    qd = [7] + qd[:8] + [23] + qd[8:]
                # build emission plan: per pair, its two ACT quads, then some
                # DVE quads, then the pair's DR matmul
                prod = []
                di = 0
                for i in range(npairs):
                    prod.append(("ar", i))
                    take = round((i + 1) * len(qd) / npairs) - di
                    for _ in range(take):
                        prod.append(("d", qd[di]))
                        di += 1
                # consumers (scores MMs) lag producers by one slot so the PE
                # queue head never waits on a just-issued relu
                plan = []
                pend = []
                for p in prod:
                    plan.append(p)
                    if pend:
                        plan.append(pend.pop(0))
                    pend.append(("drmm", p[1]) if p[0] == "ar"
                                else ("dmm", p[1]))
                plan.extend(pend)

                n_sc_mms = len(qd) + npairs
                ps_sc = sc_ps.tile([NA, 4 * NB], F32, tag="sc")
                sc_i = 0

                y8_tiles = {}
                t4_tiles = {}
                for kind, v in plan:
                    if kind == "ar":
                        i = v
                        y8 = y8_pool.tile([128, 1024], FP8, tag="y8")
                        y8_tiles[i] = y8
                        for half, q in ((0, i), (1, i + 16)):
                            ps_y4 = y4_ps.tile([128, 512], F32, tag="y4")
                            nc.tensor.matmul(
                                ps_y4, lhsT=ident_bf, rhs=xb1_b,
                                start=True, stop=False, skip_group_check=True)
                            comb = ident_bf[:, q:q + 97:32].unsqueeze(2) \
                                .broadcast_to((128, 4, 128))
                            nc.tensor.matmul(
                                ps_y4, lhsT=xar_bf, rhs=comb,
                                start=False, stop=True, skip_group_check=True)
                            nc.scalar.activation(
                                out=y8[:, half * 512:(half + 1) * 512],
                                in_=ps_y4, func=AF.Relu)
                    elif kind == "d":
                        q = v
                        t4 = t_pool.tile([D, 4 * NB], BF16, tag="t")
                        t4_tiles[q] = t4
                        for j in range(4):
                            n = q + 32 * j
                            nc.vector.tensor_scalar(
                                out=t4[:, j * NB:(j + 1) * NB], in0=xb1_bf,
                                scalar1=xa_sb[:, n:n + 1], scalar2=0.0,
                                op0=OP.add, op1=OP.max,
                            )
                    elif kind == "dmm":
                        q = v
                        t4 = t4_tiles.pop(q)
                        nc.tensor.matmul(
                            ps_sc, lhsT=w2s_sb[:, 32 - q:160 - q], rhs=t4,
                            start=(sc_i == 0), stop=(sc_i == n_sc_mms - 1),
                            skip_group_check=True,
                        )
                        sc_i += 1
                    else:  # drmm
                        i = v
                        y8 = y8_tiles.pop(i)
                        w83 = w8_sb[:, i * 256:(i + 1) * 256].rearrange(
                            "p (two m) -> p two m", two=2)
                        y83 = y8.rearrange("p (two n) -> p two n", two=2)
                        nc.tensor.matmul(
                            ps_sc, lhsT=w83, rhs=y83, perf_mode=DRM,
                            start=(sc_i == 0), stop=(sc_i == n_sc_mms - 1),
                            skip_group_check=True,
                        )
                        sc_i += 1

                # E[n, m] = exp(scores)
                e_sb = e_pool.tile([NA, NB], BF16, tag="E")
                for u in range(4):
                    nc.scalar.activation(
                        out=e_sb[32 * u:32 * (u + 1), :],
                        in_=ps_sc[32 * u:32 * (u + 1), u * NB:(u + 1) * NB],
                        func=AF.Exp,
                    )

                ps_tr = tr_ps.tile([NB, NA], BF16, tag="tr")
                nc.tensor.transpose(ps_tr, e_sb, ident_bf)
                et_sb = e_pool.tile([NB, NA], BF16, tag="Et")
                nc.scalar.copy(out=et_sb, in_=ps_tr)

                ps_ab = ab_ps.tile([NA, 272], F32, tag="ab")
                ps_a = ps_ab[:, 0:129]
                nc.tensor.matmul(ps_a, lhsT=et_sb, rhs=hbEb_sb, start=True, stop=True)
                ps_b = ps_ab[:, 136:265]
                nc.tensor.matmul(ps_b, lhsT=e_sb, rhs=haEb_sb, start=True, stop=True)

                ra = r_pool.tile([NA, 1], F32, tag="r")
                nc.vector.reciprocal(out=ra, in_=ps_a[:, D:D + 1])
                outa = out_pool.tile([NA, D], F32, tag="oa")
                nc.vector.scalar_tensor_tensor(
                    out=outa, in0=ps_a[:, 0:D], scalar=ra[:, 0:1],
                    in1=haE_sb[:, 0:D], op0=OP.mult, op1=OP.add,
                )
                nc.sync.dma_start(out=mua[g], in_=outa)

                rb = r_pool.tile([NB, 1], F32, tag="r")
                nc.vector.reciprocal(out=rb, in_=ps_b[:, D:D + 1])
                outb = out_pool.tile([NB, D], F32, tag="ob")
                nc.vector.scalar_tensor_tensor(
                    out=outb, in0=ps_b[:, 0:D], scalar=rb[:, 0:1],
                    in1=hbE_sb[:, 0:D], op0=OP.mult, op1=OP.add,
                )
                nc.sync.dma_start(out=mub[g], in_=outb)

    nc.compile()
    return nc


def _get_program():
    if "nc" not in _CACHE:
        _CACHE["nc"] = _build_program()
    return _CACHE["nc"]


def _prep_in_maps(h_a, h_b, W1, b1, W2):
    h_a = np.asarray(h_a, dtype=np.float32)
    h_b = np.asarray(h_b, dtype=np.float32)
    W1 = np.asarray(W1, dtype=np.float32)
    b1 = np.asarray(b1, dtype=np.float32)
    W2 = np.asarray(W2, dtype=np.float32)

    w1aT = np.ascontiguousarray(W1[:, :D].T).astype(ml_dtypes.bfloat16)
    w1bT = np.ascontiguousarray(W1[:, D:].T).astype(ml_dtypes.bfloat16)
    b1c = np.ascontiguousarray(b1.reshape(D, 1))
    w2bf = W2[0].astype(ml_dtypes.bfloat16).astype(np.float32)
    comb = (np.arange(160) % 32 == 0).astype(np.float32)
    w2s_f = w2bf[:, None] * comb[None, :]
    w2s = np.ascontiguousarray(w2s_f).astype(ml_dtypes.bfloat16)
    # DR stationaries: pair i -> slot0 = comb shift for q=i, slot1 for q=i+16
    w8 = np.concatenate(
        [np.concatenate([w2s_f[:, 32 - i:160 - i], w2s_f[:, 16 - i:144 - i]],
                        axis=1) for i in range(8)], axis=1)
    w8 = np.ascontiguousarray(w8).astype(ml_dtypes.float8_e4m3)

    neg = np.full((G * NA, 1), -1.0, dtype=np.float32)

    in_maps = []
    for c in range(NCORES):
        ha = h_a[c * G * NA:(c + 1) * G * NA]
        hb = h_b[c * G * NB:(c + 1) * G * NB]
        haE = np.ascontiguousarray(np.concatenate([ha, neg], axis=1))
        hbE = np.ascontiguousarray(np.concatenate([hb, neg], axis=1))
        haT = np.ascontiguousarray(
            ha.reshape(G, NA, D).transpose(2, 0, 1).reshape(D, G * NA)
        ).astype(ml_dtypes.bfloat16)
        hbT = np.ascontiguousarray(
            hb.reshape(G, NB, D).transpose(2, 0, 1).reshape(D, G * NB)
        ).astype(ml_dtypes.bfloat16)
        in_maps.append({
            "haE": haE, "hbE": hbE, "haT": haT, "hbT": hbT,
            "haEb": haE.astype(ml_dtypes.bfloat16),
            "hbEb": hbE.astype(ml_dtypes.bfloat16),
            "w1aT": w1aT, "w1bT": w1bT, "b1c": b1c, "w2s": w2s, "w8": w8,
        })
    return in_maps


def run(h_a, h_b, W1, b1, W2, trace=False, **run_kwargs):
    nc = _get_program()
    in_maps = _prep_in_maps(h_a, h_b, W1, b1, W2)
    res = bass_utils.run_bass_kernel_spmd(
        nc, in_maps, core_ids=list(range(NCORES)), trace=trace, **run_kwargs
    )
    mu_a = np.concatenate([r["mu_a"] for r in res.results], axis=0)
    mu_b = np.concatenate([r["mu_b"] for r in res.results], axis=0)
    return (mu_a, mu_b), res


def kernel(h_a, batch_a, h_b, batch_b, W1, b1, W2, b2):
    # batch_a/batch_b encode the (equal-sized, sorted) graph partition that the
    # dense [B, n, D] view already assumes; b2 shifts scores uniformly and
    # cancels in both softmaxes.
    (mu_a, mu_b), _ = run(h_a, h_b, W1, b1, W2, trace=False)
    return mu_a, mu_b


# revision 8
# speedup vs baseline: 1.1019x; 1.0083x over previous
"""Cross-graph attention kernel for Trainium2 (8 NeuronCores, SPMD data-parallel over B).

v2: three-way engine balance. Per graph (B=32, NA=NB=D=128):
  - prep (bf16 PE): xa_T[h,n], xa_rows[n,h], xb_T[h,m]; xb1 = xb_T + b1 (bf16),
    xa_sb (f32 scalar source), xar_bf (bf16 stationary).
  - relu tensor t[n][h,m] = relu(xb1 + xa_n) produced by TWO lanes:
    * DVE lane (q in QD): 4x tensor_scalar(add col, max 0) [128,128] bf16
      -> one bf16 comb matmul per quad into the scores psum (diagonal layout).
    * ACT lane (q pairs (q, q+16)): PE builds y4 = xb1(x4) + xa rows in PSUM
      (identity stationary with broadcast moving + xar stationary with
      stride-0 comb moving over identity columns), ACT applies relu
      PSUM->fp8 [128,512]; two quads -> one fp8 DoubleRow matmul (2x K) into
      the same scores psum.
  - scores psum [128,512]: row p holds its scores at free block p//32.
  - exp via 4 partition-range ACT ops -> E bf16; transpose via PE;
    attention matmuls with [h|-1]-extended hb/ha (bf16) give numerator and
    -denominator; mu = h + num * (-1/S) as scalar_tensor_tensor.
"""

import numpy as np
import ml_dtypes

import concourse.bass as bass
import concourse.tile as tile
from concourse import bacc, mybir
from concourse import bass_utils
from concourse.masks import make_identity

F32 = mybir.dt.float32
BF16 = mybir.dt.bfloat16
FP8 = mybir.dt.float8e4
AF = mybir.ActivationFunctionType
OP = mybir.AluOpType
DRM = mybir.MatmulPerfMode.DoubleRow

B, NA, NB, D = 32, 128, 128, 128
NCORES = 8
G = B // NCORES  # graphs per core

# per-graph lane split: even graphs 8 ACT pairs, odd graphs 7 (DVE/ACT
# balance ~68/60 quads per core)
PAIRS = {0: 8, 1: 7, 2: 8, 3: 7}

_CACHE = {}


def _build_program():
    nc = bacc.Bacc(
        "TRN2",
        target_bir_lowering=False,
        debug=False,
        enable_asserts=False,
        num_devices=NCORES,
    )

    haE_d = nc.dram_tensor("haE", [G * NA, D + 1], F32, kind="ExternalInput")
    hbE_d = nc.dram_tensor("hbE", [G * NB, D + 1], F32, kind="ExternalInput")
    haEb_d = nc.dram_tensor("haEb", [G * NA, D + 1], BF16, kind="ExternalInput")
    hbEb_d = nc.dram_tensor("hbEb", [G * NB, D + 1], BF16, kind="ExternalInput")
    haT_d = nc.dram_tensor("haT", [D, G * NA], BF16, kind="ExternalInput")
    hbT_d = nc.dram_tensor("hbT", [D, G * NB], BF16, kind="ExternalInput")
    w1aT_d = nc.dram_tensor("w1aT", [D, D], BF16, kind="ExternalInput")
    w1bT_d = nc.dram_tensor("w1bT", [D, D], BF16, kind="ExternalInput")
    b1_d = nc.dram_tensor("b1c", [D, 1], F32, kind="ExternalInput")
    w2s_d = nc.dram_tensor("w2s", [D, 160], BF16, kind="ExternalInput")
    w8_d = nc.dram_tensor("w8", [D, 8 * 256], FP8, kind="ExternalInput")
    mua_d = nc.dram_tensor("mu_a", [G * NA, D], F32, kind="ExternalOutput")
    mub_d = nc.dram_tensor("mu_b", [G * NB, D], F32, kind="ExternalOutput")

    haE = haE_d.ap().rearrange("(g n) c -> g n c", g=G)
    hbE = hbE_d.ap().rearrange("(g n) c -> g n c", g=G)
    haT = haT_d.ap()
    hbT = hbT_d.ap()
    haEb = haEb_d.ap().rearrange("(g n) c -> g n c", g=G)
    hbEb = hbEb_d.ap().rearrange("(g n) c -> g n c", g=G)
    mua = mua_d.ap().rearrange("(g n) c -> g n c", g=G)
    mub = mub_d.ap().rearrange("(g n) c -> g n c", g=G)

    with tile.TileContext(nc) as tc:
        with (
            tc.tile_pool(name="consts", bufs=1) as consts,
            tc.tile_pool(name="io", bufs=3) as io,
            tc.tile_pool(name="gp", bufs=2) as gp,
            tc.tile_pool(name="t", bufs=8) as t_pool,
            tc.tile_pool(name="y8p", bufs=3) as y8_pool,
            tc.tile_pool(name="ee", bufs=2) as e_pool,
            tc.tile_pool(name="r", bufs=4) as r_pool,
            tc.tile_pool(name="outs", bufs=4) as out_pool,
            tc.tile_pool(name="prep_ps", bufs=1, space="PSUM") as prep_ps,
            tc.tile_pool(name="y4_ps", bufs=3, space="PSUM") as y4_ps,
            tc.tile_pool(name="sc_ps", bufs=2, space="PSUM") as sc_ps,
            tc.tile_pool(name="tr_ps", bufs=1, space="PSUM") as tr_ps,
            tc.tile_pool(name="ab_ps", bufs=1, space="PSUM") as ab_ps,
        ):
            ident_bf = consts.tile([128, 128], BF16)
            make_identity(nc, ident_bf)
            w1aT_sb = consts.tile([D, D], BF16)
            nc.sync.dma_start(out=w1aT_sb, in_=w1aT_d.ap())
            w1bT_sb = consts.tile([D, D], BF16)
            nc.sync.dma_start(out=w1bT_sb, in_=w1bT_d.ap())
            b1_sb = consts.tile([D, 1], F32)
            nc.sync.dma_start(out=b1_sb, in_=b1_d.ap())
            w2s_sb = consts.tile([D, 160], BF16)
            nc.sync.dma_start(out=w2s_sb, in_=w2s_d.ap())
            w8_sb = consts.tile([D, 8 * 256], FP8)
            nc.sync.dma_start(out=w8_sb, in_=w8_d.ap())

            for g in range(G):
                # phase-1-critical DMAs first
                haT_sb = io.tile([D, NA], BF16, tag="haT")
                nc.sync.dma_start(out=haT_sb, in_=haT[:, g * NA:(g + 1) * NA])
                hbT_sb = io.tile([D, NB], BF16, tag="hbT")
                nc.sync.dma_start(out=hbT_sb, in_=hbT[:, g * NB:(g + 1) * NB])
                haE_sb = io.tile([NA, D + 1], F32, tag="haE")
                nc.sync.dma_start(out=haE_sb, in_=haE[g])
                hbE_sb = io.tile([NB, D + 1], F32, tag="hbE")
                nc.sync.dma_start(out=hbE_sb, in_=hbE[g])
                haEb_sb = io.tile([NA, D + 1], BF16, tag="haEb")
                nc.sync.dma_start(out=haEb_sb, in_=haEb[g])
                hbEb_sb = io.tile([NB, D + 1], BF16, tag="hbEb")
                nc.sync.dma_start(out=hbEb_sb, in_=hbEb[g])

                # prep matmuls (bf16)
                ps_prep = prep_ps.tile([D, 384], F32, tag="prep")
                ps_xa = ps_prep[:, 0:128]
                nc.tensor.matmul(ps_xa, lhsT=w1aT_sb, rhs=haT_sb, start=True, stop=True)
                ps_xar = ps_prep[:, 128:256]
                nc.tensor.matmul(ps_xar, lhsT=haT_sb, rhs=w1aT_sb, start=True, stop=True)
                ps_xb = ps_prep[:, 256:384]
                nc.tensor.matmul(ps_xb, lhsT=w1bT_sb, rhs=hbT_sb, start=True, stop=True)

                xa_sb = gp.tile([D, NA], F32, tag="xa")
                nc.vector.tensor_copy(out=xa_sb, in_=ps_xa)
                xb1_bf = gp.tile([D, NB], BF16, tag="xb1")
                nc.vector.tensor_scalar(
                    out=xb1_bf, in0=ps_xb, scalar1=b1_sb[:, 0:1], scalar2=None,
                    op0=OP.add,
                )
                xar_bf = gp.tile([NA, D], BF16, tag="xar")
                nc.scalar.copy(out=xar_bf, in_=ps_xar)

                xb1_b = xb1_bf[:, :].unsqueeze(1).broadcast_to((128, 4, 128))

                npairs = PAIRS[g]
                qd = [q for q in range(8, 16)] + [q for q in range(24, 32)]
                if npairs == 7:
                    qd = [7] + qd[:8] + [23] + qd[8:]
                # build emission plan: per pair, its two ACT quads, then some
                # DVE quads, then the pair's DR matmul
                prod = []
                di = 0
                for i in range(npairs):
                    prod.append(("ar", i))
                    take = round((i + 1) * len(qd) / npairs) - di
                    for _ in range(take):
                        prod.append(("d", qd[di]))
                        di += 1
                # consumers (scores MMs) lag producers by one slot so the PE
                # queue head never waits on a just-issued relu
                plan = []
                pend = []
                for p in prod:
                    plan.append(p)
                    if pend:
                        plan.append(pend.pop(0))
                    pend.append(("drmm", p[1]) if p[0] == "ar"
                                else ("dmm", p[1]))
                plan.extend(pend)

                n_sc_mms = len(qd) + npairs
                ps_sc = sc_ps.tile([NA, 4 * NB], F32, tag="sc")
                sc_i = 0

                y8_tiles = {}
                t4_tiles = {}
                for kind, v in plan:
                    if kind == "ar":
                        i = v
                        y8 = y8_pool.tile([128, 1024], FP8, tag="y8")
                        y8_tiles[i] = y8
                        ys = []
                        for half, q in ((0, i), (1, i + 16)):
                            ps_y4 = y4_ps.tile([128, 512], F32, tag="y4")
                            nc.tensor.matmul(
                                ps_y4, lhsT=ident_bf, rhs=xb1_b,
                                start=True, stop=False, skip_group_check=True)
                            comb = ident_bf[:, q:q + 97:32].unsqueeze(2) \
                                .broadcast_to((128, 4, 128))
                            nc.tensor.matmul(
                                ps_y4, lhsT=xar_bf, rhs=comb,
                                start=False, stop=True, skip_group_check=True)
                            ys.append((half, ps_y4))
                        for half, ps_y4 in ys:
                            nc.scalar.activation(
                                out=y8[:, half * 512:(half + 1) * 512],
                                in_=ps_y4, func=AF.Relu)
                    elif kind == "d":
                        q = v
                        t4 = t_pool.tile([D, 4 * NB], BF16, tag="t")
                        t4_tiles[q] = t4
                        for j in range(4):
                            n = q + 32 * j
                            nc.vector.tensor_scalar(
                                out=t4[:, j * NB:(j + 1) * NB], in0=xb1_bf,
                                scalar1=xa_sb[:, n:n + 1], scalar2=0.0,
                                op0=OP.add, op1=OP.max,
                            )
                    elif kind == "dmm":
                        q = v
                        t4 = t4_tiles.pop(q)
                        nc.tensor.matmul(
                            ps_sc, lhsT=w2s_sb[:, 32 - q:160 - q], rhs=t4,
                            start=(sc_i == 0), stop=(sc_i == n_sc_mms - 1),
                            skip_group_check=True,
                        )
                        sc_i += 1
                    else:  # drmm
                        i = v
                        y8 = y8_tiles.pop(i)
                        w83 = w8_sb[:, i * 256:(i + 1) * 256].rearrange(
                            "p (two m) -> p two m", two=2)
                        y83 = y8.rearrange("p (two n) -> p two n", two=2)
                        nc.tensor.matmul(
                            ps_sc, lhsT=w83, rhs=y83, perf_mode=DRM,
                            start=(sc_i == 0), stop=(sc_i == n_sc_mms - 1),
                            skip_group_check=True,
                        )
                        sc_i += 1

                # E[n, m] = exp(scores)
                e_sb = e_pool.tile([NA, NB], BF16, tag="E")
                for u in range(4):
                    nc.scalar.activation(
                        out=e_sb[32 * u:32 * (u + 1), :],
                        in_=ps_sc[32 * u:32 * (u + 1), u * NB:(u + 1) * NB],
                        func=AF.Exp,
                    )

                ps_tr = tr_ps.tile([NB, NA], BF16, tag="tr")
                nc.tensor.transpose(ps_tr, e_sb, ident_bf)
                et_sb = e_pool.tile([NB, NA], BF16, tag="Et")
                nc.scalar.copy(out=et_sb, in_=ps_tr)

                ps_ab = ab_ps.tile([NA, 272], F32, tag="ab")
                ps_a = ps_ab[:, 0:129]
                nc.tensor.matmul(ps_a, lhsT=et_sb, rhs=hbEb_sb, start=True, stop=True)
                ps_b = ps_ab[:, 136:265]
                nc.tensor.matmul(ps_b, lhsT=e_sb, rhs=haEb_sb, start=True, stop=True)

                ra = r_pool.tile([NA, 1], F32, tag="r")
                nc.vector.reciprocal(out=ra, in_=ps_a[:, D:D + 1])
                outa = out_pool.tile([NA, D], F32, tag="oa")
                nc.vector.scalar_tensor_tensor(
                    out=outa, in0=ps_a[:, 0:D], scalar=ra[:, 0:1],
                    in1=haE_sb[:, 0:D], op0=OP.mult, op1=OP.add,
                )
                nc.sync.dma_start(out=mua[g], in_=outa)

                rb = r_pool.tile([NB, 1], F32, tag="r")
                nc.vector.reciprocal(out=rb, in_=ps_b[:, D:D + 1])
                outb = out_pool.tile([NB, D], F32, tag="ob")
                nc.vector.scalar_tensor_tensor(
                    out=outb, in0=ps_b[:, 0:D], scalar=rb[:, 0:1],
                    in1=hbE_sb[:, 0:D], op0=OP.mult, op1=OP.add,
                )
                nc.sync.dma_start(out=mub[g], in_=outb)

    nc.compile()
    return nc


def _get_program():
    if "nc" not in _CACHE:
        _CACHE["nc"] = _build_program()
    return _CACHE["nc"]


def _prep_in_maps(h_a, h_b, W1, b1, W2):
    h_a = np.asarray(h_a, dtype=np.float32)
    h_b = np.asarray(h_b, dtype=np.float32)
    W1 = np.asarray(W1, dtype=np.float32)
    b1 = np.asarray(b1, dtype=np.float32)
    W2 = np.asarray(W2, dtype=np.float32)

    w1aT = np.ascontiguousarray(W1[:, :D].T).astype(ml_dtypes.bfloat16)
    w1bT = np.ascontiguousarray(W1[:, D:].T).astype(ml_dtypes.bfloat16)
    b1c = np.ascontiguousarray(b1.reshape(D, 1))
    w2bf = W2[0].astype(ml_dtypes.bfloat16).astype(np.float32)
    comb = (np.arange(160) % 32 == 0).astype(np.float32)
    w2s_f = w2bf[:, None] * comb[None, :]
    w2s = np.ascontiguousarray(w2s_f).astype(ml_dtypes.bfloat16)
    # DR stationaries: pair i -> slot0 = comb shift for q=i, slot1 for q=i+16
    w8 = np.concatenate(
        [np.concatenate([w2s_f[:, 32 - i:160 - i], w2s_f[:, 16 - i:144 - i]],
                        axis=1) for i in range(8)], axis=1)
    w8 = np.ascontiguousarray(w8).astype(ml_dtypes.float8_e4m3)

    neg = np.full((G * NA, 1), -1.0, dtype=np.float32)

    in_maps = []
    for c in range(NCORES):
        ha = h_a[c * G * NA:(c + 1) * G * NA]
        hb = h_b[c * G * NB:(c + 1) * G * NB]
        haE = np.ascontiguousarray(np.concatenate([ha, neg], axis=1))
        hbE = np.ascontiguousarray(np.concatenate([hb, neg], axis=1))
        haT = np.ascontiguousarray(
            ha.reshape(G, NA, D).transpose(2, 0, 1).reshape(D, G * NA)
        ).astype(ml_dtypes.bfloat16)
        hbT = np.ascontiguousarray(
            hb.reshape(G, NB, D).transpose(2, 0, 1).reshape(D, G * NB)
        ).astype(ml_dtypes.bfloat16)
        in_maps.append({
            "haE": haE, "hbE": hbE, "haT": haT, "hbT": hbT,
            "haEb": haE.astype(ml_dtypes.bfloat16),
            "hbEb": hbE.astype(ml_dtypes.bfloat16),
            "w1aT": w1aT, "w1bT": w1bT, "b1c": b1c, "w2s": w2s, "w8": w8,
        })
    return in_maps


def run(h_a, h_b, W1, b1, W2, trace=False, **run_kwargs):
    nc = _get_program()
    in_maps = _prep_in_maps(h_a, h_b, W1, b1, W2)
    res = bass_utils.run_bass_kernel_spmd(
        nc, in_maps, core_ids=list(range(NCORES)), trace=trace, **run_kwargs
    )
    mu_a = np.concatenate([r["mu_a"] for r in res.results], axis=0)
    mu_b = np.concatenate([r["mu_b"] for r in res.results], axis=0)
    return (mu_a, mu_b), res


def kernel(h_a, batch_a, h_b, batch_b, W1, b1, W2, b2):
    # batch_a/batch_b encode the (equal-sized, sorted) graph partition that the
    # dense [B, n, D] view already assumes; b2 shifts scores uniformly and
    # cancels in both softmaxes.
    (mu_a, mu_b), _ = run(h_a, h_b, W1, b1, W2, trace=False)
    return mu_a, mu_b


# revision 9
# speedup vs baseline: 1.1034x; 1.0013x over previous
"""Cross-graph attention kernel for Trainium2 (8 NeuronCores, SPMD data-parallel over B).

v2: three-way engine balance. Per graph (B=32, NA=NB=D=128):
  - prep (bf16 PE): xa_T[h,n], xa_rows[n,h], xb_T[h,m]; xb1 = xb_T + b1 (bf16),
    xa_sb (f32 scalar source), xar_bf (bf16 stationary).
  - relu tensor t[n][h,m] = relu(xb1 + xa_n) produced by TWO lanes:
    * DVE lane (q in QD): 4x tensor_scalar(add col, max 0) [128,128] bf16
      -> one bf16 comb matmul per quad into the scores psum (diagonal layout).
    * ACT lane (q pairs (q, q+16)): PE builds y4 = xb1(x4) + xa rows in PSUM
      (identity stationary with broadcast moving + xar stationary with
      stride-0 comb moving over identity columns), ACT applies relu
      PSUM->fp8 [128,512]; two quads -> one fp8 DoubleRow matmul (2x K) into
      the same scores psum.
  - scores psum [128,512]: row p holds its scores at free block p//32.
  - exp via 4 partition-range ACT ops -> E bf16; transpose via PE;
    attention matmuls with [h|-1]-extended hb/ha (bf16) give numerator and
    -denominator; mu = h + num * (-1/S) as scalar_tensor_tensor.
"""

import numpy as np
import ml_dtypes

import concourse.bass as bass
import concourse.tile as tile
from concourse import bacc, mybir
from concourse import bass_utils
from concourse.masks import make_identity

F32 = mybir.dt.float32
BF16 = mybir.dt.bfloat16
FP8 = mybir.dt.float8e4
AF = mybir.ActivationFunctionType
OP = mybir.AluOpType
DRM = mybir.MatmulPerfMode.DoubleRow

B, NA, NB, D = 32, 128, 128, 128
NCORES = 8
G = B // NCORES  # graphs per core

# per-graph lane split: even graphs 8 ACT pairs, odd graphs 7 (DVE/ACT
# balance ~68/60 quads per core)
PAIRS = {0: 8, 1: 7, 2: 8, 3: 7}

_CACHE = {}


def _build_program():
    nc = bacc.Bacc(
        "TRN2",
        target_bir_lowering=False,
        debug=False,
        enable_asserts=False,
        num_devices=NCORES,
    )

    haE_d = nc.dram_tensor("haE", [G * NA, D + 1], F32, kind="ExternalInput")
    hbE_d = nc.dram_tensor("hbE", [G * NB, D + 1], F32, kind="ExternalInput")
    haEb_d = nc.dram_tensor("haEb", [G * NA, D + 1], BF16, kind="ExternalInput")
    hbEb_d = nc.dram_tensor("hbEb", [G * NB, D + 1], BF16, kind="ExternalInput")
    haT_d = nc.dram_tensor("haT", [D, G * NA], BF16, kind="ExternalInput")
    hbT_d = nc.dram_tensor("hbT", [D, G * NB], BF16, kind="ExternalInput")
    w1aT_d = nc.dram_tensor("w1aT", [D, D], BF16, kind="ExternalInput")
    w1bT_d = nc.dram_tensor("w1bT", [D, D], BF16, kind="ExternalInput")
    b1_d = nc.dram_tensor("b1c", [D, 1], F32, kind="ExternalInput")
    w2s_d = nc.dram_tensor("w2s", [D, 160], BF16, kind="ExternalInput")
    w8_d = nc.dram_tensor("w8", [D, 8 * 256], FP8, kind="ExternalInput")
    mua_d = nc.dram_tensor("mu_a", [G * NA, D], F32, kind="ExternalOutput")
    mub_d = nc.dram_tensor("mu_b", [G * NB, D], F32, kind="ExternalOutput")

    haE = haE_d.ap().rearrange("(g n) c -> g n c", g=G)
    hbE = hbE_d.ap().rearrange("(g n) c -> g n c", g=G)
    haT = haT_d.ap()
    hbT = hbT_d.ap()
    haEb = haEb_d.ap().rearrange("(g n) c -> g n c", g=G)
    hbEb = hbEb_d.ap().rearrange("(g n) c -> g n c", g=G)
    mua = mua_d.ap().rearrange("(g n) c -> g n c", g=G)
    mub = mub_d.ap().rearrange("(g n) c -> g n c", g=G)

    with tile.TileContext(nc) as tc:
        with (
            tc.tile_pool(name="consts", bufs=1) as consts,
            tc.tile_pool(name="io", bufs=3) as io,
            tc.tile_pool(name="gp", bufs=2) as gp,
            tc.tile_pool(name="t", bufs=8) as t_pool,
            tc.tile_pool(name="y8p", bufs=3) as y8_pool,
            tc.tile_pool(name="ee", bufs=2) as e_pool,
            tc.tile_pool(name="r", bufs=4) as r_pool,
            tc.tile_pool(name="outs", bufs=4) as out_pool,
            tc.tile_pool(name="prep_ps", bufs=1, space="PSUM") as prep_ps,
            tc.tile_pool(name="y4_ps", bufs=3, space="PSUM") as y4_ps,
            tc.tile_pool(name="sc_ps", bufs=2, space="PSUM") as sc_ps,
            tc.tile_pool(name="tr_ps", bufs=1, space="PSUM") as tr_ps,
            tc.tile_pool(name="ab_ps", bufs=1, space="PSUM") as ab_ps,
        ):
            ident_bf = consts.tile([128, 128], BF16)
            make_identity(nc, ident_bf)
            w1aT_sb = consts.tile([D, D], BF16)
            nc.gpsimd.dma_start(out=w1aT_sb, in_=w1aT_d.ap())
            w1bT_sb = consts.tile([D, D], BF16)
            nc.gpsimd.dma_start(out=w1bT_sb, in_=w1bT_d.ap())
            b1_sb = consts.tile([D, 1], F32)
            nc.gpsimd.dma_start(out=b1_sb, in_=b1_d.ap())
            w2s_sb = consts.tile([D, 160], BF16)
            nc.sync.dma_start(out=w2s_sb, in_=w2s_d.ap())
            w8_sb = consts.tile([D, 8 * 256], FP8)
            nc.sync.dma_start(out=w8_sb, in_=w8_d.ap())

            for g in range(G):
                # phase-1-critical DMAs first
                haT_sb = io.tile([D, NA], BF16, tag="haT")
                nc.gpsimd.dma_start(out=haT_sb, in_=haT[:, g * NA:(g + 1) * NA])
                hbT_sb = io.tile([D, NB], BF16, tag="hbT")
                nc.gpsimd.dma_start(out=hbT_sb, in_=hbT[:, g * NB:(g + 1) * NB])
                haE_sb = io.tile([NA, D + 1], F32, tag="haE")
                nc.gpsimd.dma_start(out=haE_sb, in_=haE[g])
                hbE_sb = io.tile([NB, D + 1], F32, tag="hbE")
                nc.gpsimd.dma_start(out=hbE_sb, in_=hbE[g])
                haEb_sb = io.tile([NA, D + 1], BF16, tag="haEb")
                nc.gpsimd.dma_start(out=haEb_sb, in_=haEb[g])
                hbEb_sb = io.tile([NB, D + 1], BF16, tag="hbEb")
                nc.gpsimd.dma_start(out=hbEb_sb, in_=hbEb[g])

                # prep matmuls (bf16)
                ps_prep = prep_ps.tile([D, 384], F32, tag="prep")
                ps_xa = ps_prep[:, 0:128]
                nc.tensor.matmul(ps_xa, lhsT=w1aT_sb, rhs=haT_sb, start=True, stop=True)
                ps_xar = ps_prep[:, 128:256]
                nc.tensor.matmul(ps_xar, lhsT=haT_sb, rhs=w1aT_sb, start=True, stop=True)
                ps_xb = ps_prep[:, 256:384]
                nc.tensor.matmul(ps_xb, lhsT=w1bT_sb, rhs=hbT_sb, start=True, stop=True)

                xa_sb = gp.tile([D, NA], F32, tag="xa")
                nc.vector.tensor_copy(out=xa_sb, in_=ps_xa)
                xb1_bf = gp.tile([D, NB], BF16, tag="xb1")
                nc.vector.tensor_scalar(
                    out=xb1_bf, in0=ps_xb, scalar1=b1_sb[:, 0:1], scalar2=None,
                    op0=OP.add,
                )
                xar_bf = gp.tile([NA, D], BF16, tag="xar")
                nc.scalar.copy(out=xar_bf, in_=ps_xar)

                xb1_b = xb1_bf[:, :].unsqueeze(1).broadcast_to((128, 4, 128))

                npairs = PAIRS[g]
                qd = [q for q in range(8, 16)] + [q for q in range(24, 32)]
                if npairs == 7:
                    qd = [7] + qd[:8] + [23] + qd[8:]
                # build emission plan: per pair, its two ACT quads, then some
                # DVE quads, then the pair's DR matmul
                prod = []
                di = 0
                for i in range(npairs):
                    prod.append(("ar", i))
                    take = round((i + 1) * len(qd) / npairs) - di
                    for _ in range(take):
                        prod.append(("d", qd[di]))
                        di += 1
                # consumers (scores MMs) lag producers by one slot so the PE
                # queue head never waits on a just-issued relu
                plan = []
                pend = []
                for p in prod:
                    plan.append(p)
                    if pend:
                        plan.append(pend.pop(0))
                    pend.append(("drmm", p[1]) if p[0] == "ar"
                                else ("dmm", p[1]))
                plan.extend(pend)

                n_sc_mms = len(qd) + npairs
                ps_sc = sc_ps.tile([NA, 4 * NB], F32, tag="sc")
                sc_i = 0

                y8_tiles = {}
                t4_tiles = {}
                for kind, v in plan:
                    if kind == "ar":
                        i = v
                        y8 = y8_pool.tile([128, 1024], FP8, tag="y8")
                        y8_tiles[i] = y8
                        ys = []
                        for half, q in ((0, i), (1, i + 16)):
                            ps_y4 = y4_ps.tile([128, 512], F32, tag="y4")
                            nc.tensor.matmul(
                                ps_y4, lhsT=ident_bf, rhs=xb1_b,
                                start=True, stop=False, skip_group_check=True)
                            comb = ident_bf[:, q:q + 97:32].unsqueeze(2) \
                                .broadcast_to((128, 4, 128))
                            nc.tensor.matmul(
                                ps_y4, lhsT=xar_bf, rhs=comb,
                                start=False, stop=True, skip_group_check=True)
                            ys.append((half, ps_y4))
                        for half, ps_y4 in ys:
                            nc.scalar.activation(
                                out=y8[:, half * 512:(half + 1) * 512],
                                in_=ps_y4, func=AF.Relu)
                    elif kind == "d":
                        q = v
                        t4 = t_pool.tile([D, 4 * NB], BF16, tag="t")
                        t4_tiles[q] = t4
                        for j in range(4):
                            n = q + 32 * j
                            nc.vector.tensor_scalar(
                                out=t4[:, j * NB:(j + 1) * NB], in0=xb1_bf,
                                scalar1=xa_sb[:, n:n + 1], scalar2=0.0,
                                op0=OP.add, op1=OP.max,
                            )
                    elif kind == "dmm":
                        q = v
                        t4 = t4_tiles.pop(q)
                        nc.tensor.matmul(
                            ps_sc, lhsT=w2s_sb[:, 32 - q:160 - q], rhs=t4,
                            start=(sc_i == 0), stop=(sc_i == n_sc_mms - 1),
                            skip_group_check=True,
                        )
                        sc_i += 1
                    else:  # drmm
                        i = v
                        y8 = y8_tiles.pop(i)
                        w83 = w8_sb[:, i * 256:(i + 1) * 256].rearrange(
                            "p (two m) -> p two m", two=2)
                        y83 = y8.rearrange("p (two n) -> p two n", two=2)
                        nc.tensor.matmul(
                            ps_sc, lhsT=w83, rhs=y83, perf_mode=DRM,
                            start=(sc_i == 0), stop=(sc_i == n_sc_mms - 1),
                            skip_group_check=True,
                        )
                        sc_i += 1

                # E[n, m] = exp(scores)
                e_sb = e_pool.tile([NA, NB], BF16, tag="E")
                for u in range(4):
                    nc.scalar.activation(
                        out=e_sb[32 * u:32 * (u + 1), :],
                        in_=ps_sc[32 * u:32 * (u + 1), u * NB:(u + 1) * NB],
                        func=AF.Exp,
                    )

                ps_tr = tr_ps.tile([NB, NA], BF16, tag="tr")
                nc.tensor.transpose(ps_tr, e_sb, ident_bf)
                et_sb = e_pool.tile([NB, NA], BF16, tag="Et")
                nc.scalar.copy(out=et_sb, in_=ps_tr)

                ps_ab = ab_ps.tile([NA, 272], F32, tag="ab")
                ps_a = ps_ab[:, 0:129]
                nc.tensor.matmul(ps_a, lhsT=et_sb, rhs=hbEb_sb, start=True, stop=True)
                ps_b = ps_ab[:, 136:265]
                nc.tensor.matmul(ps_b, lhsT=e_sb, rhs=haEb_sb, start=True, stop=True)

                ra = r_pool.tile([NA, 1], F32, tag="r")
                nc.vector.reciprocal(out=ra, in_=ps_a[:, D:D + 1])
                outa = out_pool.tile([NA, D], F32, tag="oa")
                nc.vector.scalar_tensor_tensor(
                    out=outa, in0=ps_a[:, 0:D], scalar=ra[:, 0:1],
                    in1=haE_sb[:, 0:D], op0=OP.mult, op1=OP.add,
                )
                nc.sync.dma_start(out=mua[g], in_=outa)

                rb = r_pool.tile([NB, 1], F32, tag="r")
                nc.vector.reciprocal(out=rb, in_=ps_b[:, D:D + 1])
                outb = out_pool.tile([NB, D], F32, tag="ob")
                nc.vector.scalar_tensor_tensor(
                    out=outb, in0=ps_b[:, 0:D], scalar=rb[:, 0:1],
                    in1=hbE_sb[:, 0:D], op0=OP.mult, op1=OP.add,
                )
                nc.sync.dma_start(out=mub[g], in_=outb)

    nc.compile()
    return nc


def _get_program():
    if "nc" not in _CACHE:
        _CACHE["nc"] = _build_program()
    return _CACHE["nc"]


def _prep_in_maps(h_a, h_b, W1, b1, W2):
    h_a = np.asarray(h_a, dtype=np.float32)
    h_b = np.asarray(h_b, dtype=np.float32)
    W1 = np.asarray(W1, dtype=np.float32)
    b1 = np.asarray(b1, dtype=np.float32)
    W2 = np.asarray(W2, dtype=np.float32)

    w1aT = np.ascontiguousarray(W1[:, :D].T).astype(ml_dtypes.bfloat16)
    w1bT = np.ascontiguousarray(W1[:, D:].T).astype(ml_dtypes.bfloat16)
    b1c = np.ascontiguousarray(b1.reshape(D, 1))
    w2bf = W2[0].astype(ml_dtypes.bfloat16).astype(np.float32)
    comb = (np.arange(160) % 32 == 0).astype(np.float32)
    w2s_f = w2bf[:, None] * comb[None, :]
    w2s = np.ascontiguousarray(w2s_f).astype(ml_dtypes.bfloat16)
    # DR stationaries: pair i -> slot0 = comb shift for q=i, slot1 for q=i+16
    w8 = np.concatenate(
        [np.concatenate([w2s_f[:, 32 - i:160 - i], w2s_f[:, 16 - i:144 - i]],
                        axis=1) for i in range(8)], axis=1)
    w8 = np.ascontiguousarray(w8).astype(ml_dtypes.float8_e4m3)

    neg = np.full((G * NA, 1), -1.0, dtype=np.float32)

    in_maps = []
    for c in range(NCORES):
        ha = h_a[c * G * NA:(c + 1) * G * NA]
        hb = h_b[c * G * NB:(c + 1) * G * NB]
        haE = np.ascontiguousarray(np.concatenate([ha, neg], axis=1))
        hbE = np.ascontiguousarray(np.concatenate([hb, neg], axis=1))
        haT = np.ascontiguousarray(
            ha.reshape(G, NA, D).transpose(2, 0, 1).reshape(D, G * NA)
        ).astype(ml_dtypes.bfloat16)
        hbT = np.ascontiguousarray(
            hb.reshape(G, NB, D).transpose(2, 0, 1).reshape(D, G * NB)
        ).astype(ml_dtypes.bfloat16)
        in_maps.append({
            "haE": haE, "hbE": hbE, "haT": haT, "hbT": hbT,
            "haEb": haE.astype(ml_dtypes.bfloat16),
            "hbEb": hbE.astype(ml_dtypes.bfloat16),
            "w1aT": w1aT, "w1bT": w1bT, "b1c": b1c, "w2s": w2s, "w8": w8,
        })
    return in_maps


def run(h_a, h_b, W1, b1, W2, trace=False, **run_kwargs):
    nc = _get_program()
    in_maps = _prep_in_maps(h_a, h_b, W1, b1, W2)
    res = bass_utils.run_bass_kernel_spmd(
        nc, in_maps, core_ids=list(range(NCORES)), trace=trace, **run_kwargs
    )
    mu_a = np.concatenate([r["mu_a"] for r in res.results], axis=0)
    mu_b = np.concatenate([r["mu_b"] for r in res.results], axis=0)
    return (mu_a, mu_b), res


def kernel(h_a, batch_a, h_b, batch_b, W1, b1, W2, b2):
    # batch_a/batch_b encode the (equal-sized, sorted) graph partition that the
    # dense [B, n, D] view already assumes; b2 shifts scores uniformly and
    # cancels in both softmaxes.
    (mu_a, mu_b), _ = run(h_a, h_b, W1, b1, W2, trace=False)
    return mu_a, mu_b
